# revision 20
# baseline (speedup 1.0000x reference)
"""Causal self-attention on 8 Trainium2 NeuronCores.

Sharding: core c handles batch b = c//2 and heads [(c%2)*8, (c%2)*8+8).
Each core computes the full QKV projection for its head slice, causal
flash-style attention, and the row-parallel w_o partial product. The two
partials per batch are summed on the host (no device collectives).

All PE matmuls run in fp16 with fp32 PSUM accumulation. Feature-major:
  x^T [D, N]            (host pre-transposed)
  Q^T, K^T [ch, N]      (GEMM with W stationary, x^T moving)
  V [N, 8*(64+64ones)]  (GEMM with x^T stationary, W moving)
  S^T [k, q] = K^T_tile.T @ Q^T  -> exp -> P^T [k, q]
  O^T|denom [128, q] = [V_h|ones64].T @ P^T   (rows 64-127 = denominator)
  y = O^T_norm.T @ W_o  (accumulated over ch tiles)

Causal masking: diagonal-straddling S^T blocks are computed only on
their live column range [delta:512]; the 128-wide triangular boundary
strip of P^T is zeroed post-exp by a DVE multiply with a constant
upper-triangular 0/1 tile. Dead columns are never touched by S, exp,
or AV, so stale PSUM/SBUF there is harmless.

The attention inner loop is exp-bound (ACT ~1.1us/k-tile vs ~0.65us of
PE work), so the QKV projections of future seq-chunks and the previous
chunk's out-projection are emitted as "filler" matmul steps woven into
the attention loop, with deadlines (QKV of chunk sc / channel-tile ct
must precede attention block (qc=sc, hp=ct)).
"""

from collections import deque

import numpy as np

B, N, D, H = 4, 2048, 1024, 16
DH = 64
N_CORES = 8
HPC = 8            # heads per core
CH = HPC * DH      # 512 channels per core
SCALE = 1.0 / 8.0  # 1/sqrt(DH)

_cached = None


def _build_program():
    from contextlib import ExitStack

    import concourse.tile as tile
    from concourse import bacc, mybir

    f16 = mybir.dt.float16
    f32 = mybir.dt.float32
    Exp = mybir.ActivationFunctionType.Exp
    Ln = mybir.ActivationFunctionType.Ln
    mult = mybir.AluOpType.mult
    add = mybir.AluOpType.add

    nc = bacc.Bacc(
        "TRN2", target_bir_lowering=False, debug=False, num_devices=N_CORES
    )

    xT_d = nc.dram_tensor("xT", [D, N], f16, kind="ExternalInput").ap()
    wq_d = nc.dram_tensor("wq", [D, CH], f16, kind="ExternalInput").ap()
    wk_d = nc.dram_tensor("wk", [D, CH], f16, kind="ExternalInput").ap()
    wv_d = nc.dram_tensor("wv", [D, CH], f16, kind="ExternalInput").ap()
    wo_d = nc.dram_tensor("wo", [CH, D], f16, kind="ExternalInput").ap()
    bq_d = nc.dram_tensor("bq", [CH, 1], f32, kind="ExternalInput").ap()
    bk_d = nc.dram_tensor("bk", [CH, 1], f32, kind="ExternalInput").ap()
    bv_d = nc.dram_tensor("bvb", [128, CH], f32, kind="ExternalInput").ap()
    bo_d = nc.dram_tensor("bob", [128, D], f32, kind="ExternalInput").ap()
    M2_d = nc.dram_tensor("M2", [128, 256], f16, kind="ExternalInput").ap()
    y_d = nc.dram_tensor("y", [N, D], f32, kind="ExternalOutput").ap()

    with tile.TileContext(nc) as tc, ExitStack() as ctx:
        const = ctx.enter_context(tc.tile_pool(name="const", bufs=1))
        actp = ctx.enter_context(tc.tile_pool(name="actp", bufs=1))
        work = ctx.enter_context(tc.tile_pool(name="work", bufs=3))
        ps_sp = ctx.enter_context(tc.tile_pool(name="ps_sp", bufs=2, space="PSUM"))
        ps_av = ctx.enter_context(tc.tile_pool(name="ps_av", bufs=2, space="PSUM"))
        ps_fl = ctx.enter_context(tc.tile_pool(name="ps_fl", bufs=2, space="PSUM"))

        # ---- constants / weights into SBUF (4 DMA queues, first-need order)
        wq = [const.tile([128, CH], f16, tag=f"wq{i}", name=f"wq{i}") for i in range(8)]
        wk = [const.tile([128, CH], f16, tag=f"wk{i}", name=f"wk{i}") for i in range(8)]
        wv = [const.tile([128, CH], f16, tag=f"wv{i}", name=f"wv{i}") for i in range(8)]
        xt = [[const.tile([128, 512], f16, tag=f"xt{i}_{sc}", name=f"xt{i}_{sc}")
               for sc in range(4)] for i in range(8)]
        # V[st]: per head h, cols [128h:128h+64] = V_h, [128h+64:128h+128] = 1.0
        V = [actp.tile([128, 1024], f16, tag=f"v{st}", name=f"v{st}")
             for st in range(16)]
        ones_t = const.tile([128, 64], f16, tag="ones", name="ones_t")
        nc.vector.memset(ones_t[:], 1.0)
        engs = [nc.sync, nc.gpsimd]
        _ei = [0]

        def dma_in(dst, src):
            engs[_ei[0] % len(engs)].dma_start(dst, src)
            _ei[0] += 1

        for i in range(8):
            dma_in(wk[i][:], wk_d[i * 128 : (i + 1) * 128, :])
            dma_in(xt[i][0][:], xT_d[i * 128 : (i + 1) * 128, 0:512])
        bq = [const.tile([128, 1], f32, tag=f"bq{j}", name=f"bq{j}") for j in range(4)]
        bk = [const.tile([128, 1], f32, tag=f"bk{j}", name=f"bk{j}") for j in range(4)]
        for j in range(4):
            dma_in(bq[j][:], bq_d[j * 128 : (j + 1) * 128, :])
            dma_in(bk[j][:], bk_d[j * 128 : (j + 1) * 128, :])
        for i in range(8):
            dma_in(wq[i][:], wq_d[i * 128 : (i + 1) * 128, :])
        for i in range(8):
            dma_in(wv[i][:], wv_d[i * 128 : (i + 1) * 128, :])
        bv_t = const.tile([128, CH], f32, tag="bvb", name="bvb")
        dma_in(bv_t[:], bv_d[:])
        M2_t = const.tile([128, 256], f16, tag="M2", name="M2t")
        dma_in(M2_t[:], M2_d[:])
        for sc in range(1, 4):
            for i in range(8):
                dma_in(xt[i][sc][:],
                       xT_d[i * 128 : (i + 1) * 128, sc * 512 : (sc + 1) * 512])
        wo = [const.tile([128, D], f16, tag=f"wo{j}", name=f"wo{j}") for j in range(4)]
        for j in range(4):
            dma_in(wo[j][:], wo_d[j * 128 : (j + 1) * 128, :])
        bo_t = const.tile([128, D], f32, tag="bob", name="bob")
        dma_in(bo_t[:], bo_d[:])

        # ---- persistent activations ----
        QT = [[actp.tile([128, 512], f16, tag=f"qt{ct}_{sc}", name=f"qt{ct}_{sc}")
               for sc in range(4)] for ct in range(4)]
        KT = [[actp.tile([128, 512], f16, tag=f"kt{ct}_{sc}", name=f"kt{ct}_{sc}")
               for sc in range(4)] for ct in range(4)]
        OTn = [[actp.tile([128, 512], f16, tag=f"otn{hp}_{qc}", name=f"otn{hp}_{qc}")
                for qc in range(4)] for hp in range(4)]
        # per-qc staging for deferred softmax normalization: head-pair hp
        # occupies cols hp*512:(hp+1)*512; rows 0:64 = head h0, 64:128 = h1.
        dsb = actp.tile([128, 2048], f16, tag="dsb", name="dsb")
        otun = actp.tile([128, 2048], f16, tag="otun", name="otun")

        def emit_norm(qc):
            # 1/d = exp(-ln d) batched over all 4 head-pairs of chunk qc:
            # two ACT instructions total (one table swap pair per chunk).
            lnq = work.tile([128, 2048], f32, tag="lnq", name="lnq", bufs=1)
            rq = work.tile([128, 2048], f16, tag="rq", name="rq", bufs=1)
            nc.scalar.activation(lnq[:], dsb[:], Ln)
            nc.scalar.activation(rq[:], lnq[:], Exp, scale=-1.0)
            for hp in range(4):
                cb = hp * 512
                nc.vector.tensor_mul(OTn[hp][qc][:, :],
                                     otun[:, cb:cb + 512],
                                     rq[:, cb:cb + 512])

        # ---- filler machinery ----------------------------------------
        # Each filler entry: (deadline, steps). deadline = (qc, hp) block
        # index before which all steps must be emitted; steps are closures
        # emitting ~one 512-col matmul (~215ns of PE) each.
        fq = deque()
        credit = [0.0]

        def kt_group(ct, sc):
            p = ps_fl.tile([128, 512], f32, tag="fl", name="pfl")
            steps = []
            for d in range(8):
                steps.append(lambda d=d, p=p, ct=ct, sc=sc: nc.tensor.matmul(
                    p[:], wk[d][:, ct * 128:(ct + 1) * 128], xt[d][sc][:],
                    start=(d == 0), stop=(d == 7), skip_group_check=True))
            steps.append(lambda p=p, ct=ct, sc=sc: nc.vector.tensor_scalar_add(
                KT[ct][sc][:], p[:], bk[ct][:]))
            return steps

        def qt_group(ct, sc):
            p = ps_fl.tile([128, 512], f32, tag="fl", name="pfl")
            steps = []
            for d in range(8):
                steps.append(lambda d=d, p=p, ct=ct, sc=sc: nc.tensor.matmul(
                    p[:], wq[d][:, ct * 128:(ct + 1) * 128], xt[d][sc][:],
                    start=(d == 0), stop=(d == 7), skip_group_check=True))
            steps.append(lambda p=p, ct=ct, sc=sc: nc.vector.tensor_scalar_add(
                QT[ct][sc][:], p[:], bq[ct][:]))
            return steps

        def v_group(stl, sc):
            st = 4 * sc + stl
            ts = slice(stl * 128, (stl + 1) * 128)
            p = ps_fl.tile([128, 512], f32, tag="fl", name="pfl")
            steps = []
            for d in range(8):
                steps.append(lambda d=d, p=p, sc=sc, ts=ts: nc.tensor.matmul(
                    p[:], xt[d][sc][:, ts], wv[d][:, :],
                    start=(d == 0), stop=(d == 7), skip_group_check=True))

            def fin(p=p, st=st):
                nc.vector.scalar_tensor_tensor(
                    V[st][:].rearrange("p (h e) -> p h e", e=128)[:, :, 0:64],
                    p[:].rearrange("p (h e) -> p h e", e=64),
                    1.0,
                    bv_t[:].rearrange("p (h e) -> p h e", e=64),
                    mult, add,
                )
            steps.append(fin)
            return steps

        def op_group(qc, stl, oc):
            st = 4 * qc + stl
            sl = slice(stl * 128, (stl + 1) * 128)
            ocs = slice(oc * 512, (oc + 1) * 512)
            yp = ps_fl.tile([128, 512], f32, tag="fl", name="yfl")
            steps = []
            for hpp in range(4):
                steps.append(lambda hpp=hpp, yp=yp, qc=qc, sl=sl, ocs=ocs:
                             nc.tensor.matmul(
                                 yp[:], OTn[hpp][qc][:, sl], wo[hpp][:, ocs],
                                 start=(hpp == 0), stop=(hpp == 3),
                                 skip_group_check=True))

            def fin(yp=yp, st=st, ocs=ocs):
                ysb = work.tile([128, 512], f32, tag="ysb", name="ysb")
                nc.vector.scalar_tensor_tensor(ysb[:], yp[:], 1.0,
                                               bo_t[:, ocs], mult, add)
                nc.sync.dma_start(y_d[st * 128:(st + 1) * 128, ocs], ysb[:])
            steps.append(fin)
            return steps

        def enqueue_qkv(sc):
            fq.append(((sc, 0, 0), kt_group(0, sc)))
            fq.append(((sc, 0, 0), qt_group(0, sc)))
            for stl in range(4):
                # V[4sc+stl] first consumed by AV(kt=4sc+stl) of block (sc,0),
                # which is emitted at iter kt+1 (or post-loop for the last kt)
                fq.append(((sc, 0, 4 * sc + stl + 1), v_group(stl, sc)))
            for ct in range(1, 4):
                fq.append(((sc, ct, 0), kt_group(ct, sc)))
                fq.append(((sc, ct, 0), qt_group(ct, sc)))

        def flush(block):
            # emit all filler steps whose deadline <= current block;
            # deadline-None (out-projection) entries stay, order preserved
            kept = []
            while fq:
                dl, steps = fq.popleft()
                if dl is not None and dl <= block:
                    for s in steps:
                        s()
                else:
                    kept.append((dl, steps))
            fq.extend(kept)

        def pop_steps():
            while credit[0] >= 215.0 and fq:
                dl, steps = fq[0]
                steps.pop(0)()
                if not steps:
                    fq.popleft()
                credit[0] -= 215.0

        # ---- attention blocks with woven fillers ----------------------
        enqueue_qkv(0)

        for qc in range(4):
            if qc < 3:
                enqueue_qkv(qc + 1)
            nkt = 4 * (qc + 1)
            for hp in range(4):
                flush((qc, hp, 0))
                h0, h1 = 2 * hp, 2 * hp + 1
                av = ps_av.tile([128, 512], f32, tag="av", name="av")
                dsum = work.tile([128, 1024], f16, tag="dsum", name="dsum",
                                 bufs=2)
                pend = []  # (kt, delta, pt) awaiting AV

                def emit_av(kt, delta, pt, av=av, h0=h0, h1=h1, nkt=nkt):
                    # both heads concurrently via 2x column tiling: head h0 on
                    # array cols 0-63 -> av rows 0:64, h1 on cols 64-127.
                    first, last = kt == 0, kt == nkt - 1
                    nc.tensor.matmul(
                        av[0:64, delta:512], V[kt][:, h0 * 128: h0 * 128 + 64],
                        pt[:, delta:512],
                        start=first, stop=last, skip_group_check=True,
                        tile_position=(0, 0))
                    nc.tensor.matmul(
                        av[64:128, delta:512], V[kt][:, h1 * 128: h1 * 128 + 64],
                        pt[:, 512 + delta:1024],
                        start=first, stop=last, skip_group_check=True,
                        tile_position=(0, 64))

                for kt in range(nkt):
                    flush((qc, hp, kt))
                    diag = kt >= 4 * qc
                    delta = 128 * kt - 512 * qc if diag else 0
                    sp = ps_sp.tile([128, 1024], f32, tag="sp", name="sp")
                    kcol = slice((kt % 4) * 128, (kt % 4) * 128 + 128)
                    nc.tensor.matmul(
                        sp[:, delta:512], KT[hp][kt // 4][0:64, kcol],
                        QT[hp][qc][0:64, delta:512],
                        start=True, stop=True, skip_group_check=True)
                    nc.tensor.matmul(
                        sp[:, 512 + delta:1024], KT[hp][kt // 4][64:128, kcol],
                        QT[hp][qc][64:128, delta:512],
                        start=True, stop=True, skip_group_check=True)
                    pt = work.tile([128, 1024], f16, tag="pt", name="pt")
                    if delta:
                        sp3 = sp[:].rearrange("p (b c) -> p b c", c=512)[:, :, delta:512]
                        pt3 = pt[:].rearrange("p (b c) -> p b c", c=512)[:, :, delta:512]
                        nc.scalar.activation(pt3, sp3, Exp, scale=SCALE)
                    else:
                        nc.scalar.activation(pt[:], sp[:], Exp, scale=SCALE)
                    if diag:
                        ptm = pt[:].rearrange("p (b c) -> p b c", c=512)[
                            :, :, delta:delta + 128]
                        m3 = M2_t[:].rearrange("p (b c) -> p b c", c=128)
                        nc.vector.tensor_tensor(ptm, ptm, m3, mult)
                    # denominator accumulation on the DVE (post-mask pt)
                    if kt == 0:
                        nc.vector.tensor_copy(dsum[:], pt[:])
                    elif delta:
                        d3 = dsum[:].rearrange("p (b c) -> p b c", c=512)[
                            :, :, delta:512]
                        p3 = pt[:].rearrange("p (b c) -> p b c", c=512)[
                            :, :, delta:512]
                        nc.vector.tensor_tensor(d3, d3, p3, add)
                    else:
                        nc.vector.tensor_tensor(dsum[:], dsum[:], pt[:], add)
                    pend.append((kt, delta, pt))
                    live = 2 * (512 - delta)
                    credit[0] = min(credit[0] + (264 + 0.83 * live)
                                    - (80 + 0.42 * live), 4000.0)
                    if len(pend) > 1:
                        k0, d0, p0 = pend.pop(0)
                        emit_av(k0, d0, p0)
                    pop_steps()
                flush((qc, hp, nkt))
                credit[0] = min(credit[0] + 2000.0, 4000.0)
                pop_steps()
                k0, d0, p0 = pend.pop(0)
                emit_av(k0, d0, p0)
                # denominators: col-tiled ones-matmul pair reducing dsum over
                # its 128 partitions; den rows 0:64 = head h0, 64:128 = h1
                credit[0] = min(credit[0] + 1000.0, 4000.0)
                pop_steps()
                den = ps_av.tile([128, 512], f32, tag="av", name="den")
                nc.tensor.matmul(den[0:64, :], ones_t[:], dsum[:, 0:512],
                                 start=True, stop=True, skip_group_check=True,
                                 tile_position=(0, 0))
                nc.tensor.matmul(den[64:128, :], ones_t[:], dsum[:, 512:1024],
                                 start=True, stop=True, skip_group_check=True,
                                 tile_position=(0, 64))
                # Copy av + den to SBUF (releases PSUM fast; norm is deferred
                # to the next chunk via emit_norm)
                cb = hp * 512
                nc.vector.tensor_copy(dsb[:, cb:cb + 512], den[:])
                nc.vector.tensor_copy(otun[:, cb:cb + 512], av[:])
            emit_norm(qc)
            for stl in range(4):
                for oc in range(2):
                    fq.append((None, op_group(qc, stl, oc)))

        # tail: flush any remaining fillers (out-projections)
        while fq:
            for s in fq.popleft()[1]:
                s()

    nc.compile()
    return nc


def _host_inputs(x, w_qkv, b_qkv, w_o, b_o):
    """Per-core input dicts implementing the sharding + layout prep."""
    tri = np.zeros((128, 128), np.float16)
    for r in range(128):
        tri[r, r:] = 1.0
    M2 = np.concatenate([tri, tri], axis=1)

    in_maps = []
    for c in range(N_CORES):
        b = c // 2
        hs = (c % 2) * HPC
        cols = slice(hs * DH, (hs + HPC) * DH)
        in_maps.append({
            "xT": np.ascontiguousarray(x[b].T).astype(np.float16),
            "wq": w_qkv[:, cols].astype(np.float16),
            "wk": w_qkv[:, D:][:, cols].astype(np.float16),
            "wv": w_qkv[:, 2 * D:][:, cols].astype(np.float16),
            "wo": w_o[hs * DH : (hs + HPC) * DH, :].astype(np.float16),
            "bq": b_qkv[cols].reshape(CH, 1).astype(np.float32),
            "bk": b_qkv[D:][cols].reshape(CH, 1).astype(np.float32),
            "bvb": np.tile(b_qkv[2 * D:][cols].astype(np.float32), (128, 1)),
            "bob": np.tile(b_o.astype(np.float32), (128, 1)),
            "M2": M2,
        })
    return in_maps


def kernel(x, w_qkv, b_qkv, w_o, b_o):
    global _cached
    from concourse.bass_utils import run_bass_kernel_spmd

    x = np.asarray(x)
    w_qkv = np.asarray(w_qkv)
    b_qkv = np.asarray(b_qkv)
    w_o = np.asarray(w_o)
    b_o = np.asarray(b_o)

    if _cached is None:
        _cached = _build_program()
    nc = _cached

    in_maps = _host_inputs(x, w_qkv, b_qkv, w_o, b_o)
    res = run_bass_kernel_spmd(nc, in_maps, list(range(N_CORES)))

    out = np.empty((B, N, D), np.float32)
    for b in range(B):
        out[b] = res.results[2 * b]["y"] + res.results[2 * b + 1]["y"]
    return out


# revision 21
# speedup vs baseline: 1.0066x; 1.0066x over previous
"""Causal self-attention on 8 Trainium2 NeuronCores.

Sharding: core c handles batch b = c//2 and heads [(c%2)*8, (c%2)*8+8).
Each core computes the full QKV projection for its head slice, causal
flash-style attention, and the row-parallel w_o partial product. The two
partials per batch are summed on the host (no device collectives).

All PE matmuls run in fp16 with fp32 PSUM accumulation. Feature-major:
  x^T [D, N]            (host pre-transposed)
  Q^T, K^T [ch, N]      (GEMM with W stationary, x^T moving)
  V [N, 8*(64+64ones)]  (GEMM with x^T stationary, W moving)
  S^T [k, q] = K^T_tile.T @ Q^T  -> exp -> P^T [k, q]
  O^T|denom [128, q] = [V_h|ones64].T @ P^T   (rows 64-127 = denominator)
  y = O^T_norm.T @ W_o  (accumulated over ch tiles)

Causal masking: diagonal-straddling S^T blocks are computed only on
their live column range [delta:512]; the 128-wide triangular boundary
strip of P^T is zeroed post-exp by a DVE multiply with a constant
upper-triangular 0/1 tile. Dead columns are never touched by S, exp,
or AV, so stale PSUM/SBUF there is harmless.

The attention inner loop is exp-bound (ACT ~1.1us/k-tile vs ~0.65us of
PE work), so the QKV projections of future seq-chunks and the previous
chunk's out-projection are emitted as "filler" matmul steps woven into
the attention loop, with deadlines (QKV of chunk sc / channel-tile ct
must precede attention block (qc=sc, hp=ct)).
"""

from collections import deque

import numpy as np

B, N, D, H = 4, 2048, 1024, 16
DH = 64
N_CORES = 8
HPC = 8            # heads per core
CH = HPC * DH      # 512 channels per core
SCALE = 1.0 / 8.0  # 1/sqrt(DH)

_cached = None


def _build_program():
    from contextlib import ExitStack

    import concourse.tile as tile
    from concourse import bacc, mybir

    f16 = mybir.dt.float16
    f32 = mybir.dt.float32
    Exp = mybir.ActivationFunctionType.Exp
    Ln = mybir.ActivationFunctionType.Ln
    mult = mybir.AluOpType.mult
    add = mybir.AluOpType.add

    nc = bacc.Bacc(
        "TRN2", target_bir_lowering=False, debug=False, num_devices=N_CORES
    )

    xT_d = nc.dram_tensor("xT", [D, N], f16, kind="ExternalInput").ap()
    wq_d = nc.dram_tensor("wq", [D, CH], f16, kind="ExternalInput").ap()
    wk_d = nc.dram_tensor("wk", [D, CH], f16, kind="ExternalInput").ap()
    wv_d = nc.dram_tensor("wv", [D, CH], f16, kind="ExternalInput").ap()
    wo_d = nc.dram_tensor("wo", [CH, D], f16, kind="ExternalInput").ap()
    bq_d = nc.dram_tensor("bq", [CH, 1], f32, kind="ExternalInput").ap()
    bk_d = nc.dram_tensor("bk", [CH, 1], f32, kind="ExternalInput").ap()
    bv_d = nc.dram_tensor("bvb", [128, CH], f32, kind="ExternalInput").ap()
    bo_d = nc.dram_tensor("bob", [128, D], f32, kind="ExternalInput").ap()
    M2_d = nc.dram_tensor("M2", [128, 256], f16, kind="ExternalInput").ap()
    y_d = nc.dram_tensor("y", [N, D], f32, kind="ExternalOutput").ap()

    with tile.TileContext(nc) as tc, ExitStack() as ctx:
        const = ctx.enter_context(tc.tile_pool(name="const", bufs=1))
        actp = ctx.enter_context(tc.tile_pool(name="actp", bufs=1))
        work = ctx.enter_context(tc.tile_pool(name="work", bufs=3))
        ps_sp = ctx.enter_context(tc.tile_pool(name="ps_sp", bufs=2, space="PSUM"))
        ps_av = ctx.enter_context(tc.tile_pool(name="ps_av", bufs=1, space="PSUM"))
        ps_fl = ctx.enter_context(tc.tile_pool(name="ps_fl", bufs=2, space="PSUM"))

        # ---- constants / weights into SBUF (4 DMA queues, first-need order)
        wq = [const.tile([128, CH], f16, tag=f"wq{i}", name=f"wq{i}") for i in range(8)]
        wk = [const.tile([128, CH], f16, tag=f"wk{i}", name=f"wk{i}") for i in range(8)]
        wv = [const.tile([128, CH], f16, tag=f"wv{i}", name=f"wv{i}") for i in range(8)]
        xt = [[const.tile([128, 512], f16, tag=f"xt{i}_{sc}", name=f"xt{i}_{sc}")
               for sc in range(4)] for i in range(8)]
        # V[st]: per head h, cols [128h:128h+64] = V_h, [128h+64:128h+128] = 1.0
        V = [actp.tile([128, 1024], f16, tag=f"v{st}", name=f"v{st}")
             for st in range(16)]
        for st in range(16):
            nc.vector.memset(
                V[st][:].rearrange("p (h e) -> p h e", e=128)[:, :, 64:128], 1.0)
        engs = [nc.sync, nc.gpsimd]
        _ei = [0]

        def dma_in(dst, src):
            engs[_ei[0] % len(engs)].dma_start(dst, src)
            _ei[0] += 1

        for i in range(8):
            dma_in(wk[i][:], wk_d[i * 128 : (i + 1) * 128, :])
            dma_in(xt[i][0][:], xT_d[i * 128 : (i + 1) * 128, 0:512])
        bq = [const.tile([128, 1], f32, tag=f"bq{j}", name=f"bq{j}") for j in range(4)]
        bk = [const.tile([128, 1], f32, tag=f"bk{j}", name=f"bk{j}") for j in range(4)]
        for j in range(4):
            dma_in(bq[j][:], bq_d[j * 128 : (j + 1) * 128, :])
            dma_in(bk[j][:], bk_d[j * 128 : (j + 1) * 128, :])
        for i in range(8):
            dma_in(wq[i][:], wq_d[i * 128 : (i + 1) * 128, :])
            dma_in(wv[i][:], wv_d[i * 128 : (i + 1) * 128, :])
        bv_t = const.tile([128, CH], f32, tag="bvb", name="bvb")
        dma_in(bv_t[:], bv_d[:])
        M2_t = const.tile([128, 256], f16, tag="M2", name="M2t")
        dma_in(M2_t[:], M2_d[:])
        for sc in range(1, 4):
            for i in range(8):
                dma_in(xt[i][sc][:],
                       xT_d[i * 128 : (i + 1) * 128, sc * 512 : (sc + 1) * 512])
        wo = [const.tile([128, D], f16, tag=f"wo{j}", name=f"wo{j}") for j in range(4)]
        for j in range(4):
            dma_in(wo[j][:], wo_d[j * 128 : (j + 1) * 128, :])
        bo_t = const.tile([128, D], f32, tag="bob", name="bob")
        dma_in(bo_t[:], bo_d[:])

        # ---- persistent activations ----
        QT = [[actp.tile([128, 512], f16, tag=f"qt{ct}_{sc}", name=f"qt{ct}_{sc}")
               for sc in range(4)] for ct in range(4)]
        KT = [[actp.tile([128, 512], f16, tag=f"kt{ct}_{sc}", name=f"kt{ct}_{sc}")
               for sc in range(4)] for ct in range(4)]
        OTn = [[actp.tile([128, 512], f16, tag=f"otn{hp}_{qc}", name=f"otn{hp}_{qc}")
                for qc in range(4)] for hp in range(4)]
        # per-qc staging for deferred softmax normalization: head-pair hp ->
        # rows 64*(hp//2):+64, cols (hp%2)*1024:+1024
        dsb = actp.tile([128, 2048], f16, tag="dsb", name="dsb")
        otun = actp.tile([128, 2048], f16, tag="otun", name="otun")

        def hp_stage(hp):
            rh = slice(64 * (hp // 2), 64 * (hp // 2) + 64)
            cb = (hp % 2) * 1024
            return rh, cb

        def emit_norm(qc):
            # 1/d = exp(-ln d) batched over all 4 head-pairs of chunk qc:
            # two ACT instructions total (one table swap pair per chunk).
            lnq = work.tile([128, 2048], f32, tag="lnq", name="lnq", bufs=1)
            rq = work.tile([128, 2048], f16, tag="rq", name="rq", bufs=1)
            nc.scalar.activation(lnq[:], dsb[:], Ln)
            nc.scalar.activation(rq[:], lnq[:], Exp, scale=-1.0)
            for hp in range(4):
                rh, cb = hp_stage(hp)
                nc.vector.tensor_mul(OTn[hp][qc][0:64, :],
                                     otun[rh, cb:cb + 512],
                                     rq[rh, cb:cb + 512])
                nc.vector.tensor_mul(OTn[hp][qc][64:128, :],
                                     otun[rh, cb + 512:cb + 1024],
                                     rq[rh, cb + 512:cb + 1024])

        # ---- filler machinery ----------------------------------------
        # Each filler entry: (deadline, steps). deadline = (qc, hp) block
        # index before which all steps must be emitted; steps are closures
        # emitting ~one 512-col matmul (~215ns of PE) each.
        fq = deque()
        credit = [0.0]

        def kt_group(ct, sc):
            p = ps_fl.tile([128, 512], f32, tag="fl", name="pfl")
            steps = []
            for d in range(8):
                steps.append(lambda d=d, p=p, ct=ct, sc=sc: nc.tensor.matmul(
                    p[:], wk[d][:, ct * 128:(ct + 1) * 128], xt[d][sc][:],
                    start=(d == 0), stop=(d == 7), skip_group_check=True))
            steps.append(lambda p=p, ct=ct, sc=sc: nc.vector.tensor_scalar_add(
                KT[ct][sc][:], p[:], bk[ct][:]))
            return steps

        def qt_group(ct, sc):
            p = ps_fl.tile([128, 512], f32, tag="fl", name="pfl")
            steps = []
            for d in range(8):
                steps.append(lambda d=d, p=p, ct=ct, sc=sc: nc.tensor.matmul(
                    p[:], wq[d][:, ct * 128:(ct + 1) * 128], xt[d][sc][:],
                    start=(d == 0), stop=(d == 7), skip_group_check=True))
            steps.append(lambda p=p, ct=ct, sc=sc: nc.vector.tensor_scalar_add(
                QT[ct][sc][:], p[:], bq[ct][:]))
            return steps

        def v_group(stl, sc):
            st = 4 * sc + stl
            ts = slice(stl * 128, (stl + 1) * 128)
            p = ps_fl.tile([128, 512], f32, tag="fl", name="pfl")
            steps = []
            for d in range(8):
                steps.append(lambda d=d, p=p, sc=sc, ts=ts: nc.tensor.matmul(
                    p[:], xt[d][sc][:, ts], wv[d][:, :],
                    start=(d == 0), stop=(d == 7), skip_group_check=True))

            def fin(p=p, st=st):
                nc.vector.scalar_tensor_tensor(
                    V[st][:].rearrange("p (h e) -> p h e", e=128)[:, :, 0:64],
                    p[:].rearrange("p (h e) -> p h e", e=64),
                    1.0,
                    bv_t[:].rearrange("p (h e) -> p h e", e=64),
                    mult, add,
                )
            steps.append(fin)
            return steps

        def op_group(qc, stl, oc):
            st = 4 * qc + stl
            sl = slice(stl * 128, (stl + 1) * 128)
            ocs = slice(oc * 512, (oc + 1) * 512)
            yp = ps_fl.tile([128, 512], f32, tag="fl", name="yfl")
            steps = []
            for hpp in range(4):
                steps.append(lambda hpp=hpp, yp=yp, qc=qc, sl=sl, ocs=ocs:
                             nc.tensor.matmul(
                                 yp[:], OTn[hpp][qc][:, sl], wo[hpp][:, ocs],
                                 start=(hpp == 0), stop=(hpp == 3),
                                 skip_group_check=True))

            def fin(yp=yp, st=st, ocs=ocs):
                ysb = work.tile([128, 512], f32, tag="ysb", name="ysb")
                nc.vector.scalar_tensor_tensor(ysb[:], yp[:], 1.0,
                                               bo_t[:, ocs], mult, add)
                nc.sync.dma_start(y_d[st * 128:(st + 1) * 128, ocs], ysb[:])
            steps.append(fin)
            return steps

        def enqueue_qkv(sc):
            fq.append(((sc, 0, 0), kt_group(0, sc)))
            fq.append(((sc, 0, 0), qt_group(0, sc)))
            for stl in range(4):
                # V[4sc+stl] first consumed by AV(kt=4sc+stl) of block (sc,0),
                # which is emitted at iter kt+1 (or post-loop for the last kt)
                fq.append(((sc, 0, 4 * sc + stl + 1), v_group(stl, sc)))
            for ct in range(1, 4):
                fq.append(((sc, ct, 0), kt_group(ct, sc)))
                fq.append(((sc, ct, 0), qt_group(ct, sc)))

        def flush(block):
            # emit all filler steps whose deadline <= current block;
            # deadline-None (out-projection) entries stay, order preserved
            kept = []
            while fq:
                dl, steps = fq.popleft()
                if dl is not None and dl <= block:
                    for s in steps:
                        s()
                else:
                    kept.append((dl, steps))
            fq.extend(kept)

        def pop_steps():
            while credit[0] >= 215.0 and fq:
                dl, steps = fq[0]
                steps.pop(0)()
                if not steps:
                    fq.popleft()
                credit[0] -= 215.0

        # ---- attention blocks with woven fillers ----------------------
        enqueue_qkv(0)

        for qc in range(4):
            if qc < 3:
                enqueue_qkv(qc + 1)
            nkt = 4 * (qc + 1)
            for hp in range(4):
                flush((qc, hp, 0))
                h0, h1 = 2 * hp, 2 * hp + 1
                av = ps_av.tile([128, 1024], f32, tag="av", name="av")
                pend = []  # (kt, delta, pt) awaiting AV

                def emit_av(kt, delta, pt, av=av, h0=h0, h1=h1, nkt=nkt):
                    first, last = kt == 0, kt == nkt - 1
                    nc.tensor.matmul(
                        av[:, delta:512], V[kt][:, h0 * 128: h0 * 128 + 128],
                        pt[:, delta:512],
                        start=first, stop=last, skip_group_check=True)
                    nc.tensor.matmul(
                        av[:, 512 + delta:1024], V[kt][:, h1 * 128: h1 * 128 + 128],
                        pt[:, 512 + delta:1024],
                        start=first, stop=last, skip_group_check=True)

                for kt in range(nkt):
                    flush((qc, hp, kt))
                    diag = kt >= 4 * qc
                    delta = 128 * kt - 512 * qc if diag else 0
                    sp = ps_sp.tile([128, 1024], f32, tag="sp", name="sp")
                    kcol = slice((kt % 4) * 128, (kt % 4) * 128 + 128)
                    nc.tensor.matmul(
                        sp[:, delta:512], KT[hp][kt // 4][0:64, kcol],
                        QT[hp][qc][0:64, delta:512],
                        start=True, stop=True, skip_group_check=True)
                    nc.tensor.matmul(
                        sp[:, 512 + delta:1024], KT[hp][kt // 4][64:128, kcol],
                        QT[hp][qc][64:128, delta:512],
                        start=True, stop=True, skip_group_check=True)
                    pt = work.tile([128, 1024], f16, tag="pt", name="pt", bufs=4)
                    if delta:
                        sp3 = sp[:].rearrange("p (b c) -> p b c", c=512)[:, :, delta:512]
                        pt3 = pt[:].rearrange("p (b c) -> p b c", c=512)[:, :, delta:512]
                        nc.scalar.activation(pt3, sp3, Exp, scale=SCALE)
                    else:
                        nc.scalar.activation(pt[:], sp[:], Exp, scale=SCALE)
                    if diag:
                        ptm = pt[:].rearrange("p (b c) -> p b c", c=512)[
                            :, :, delta:delta + 128]
                        m3 = M2_t[:].rearrange("p (b c) -> p b c", c=128)
                        nc.vector.tensor_tensor(ptm, ptm, m3, mult)
                    pend.append((kt, delta, pt))
                    live = 2 * (512 - delta)
                    credit[0] = min(credit[0] + (264 + 0.83 * live)
                                    - (80 + 0.625 * live), 8000.0)
                    if len(pend) > 1:
                        k0, d0, p0 = pend.pop(0)
                        emit_av(k0, d0, p0)
                    pop_steps()
                flush((qc, hp, nkt))
                credit[0] = min(credit[0] + 2000.0, 8000.0)
                pop_steps()
                k0, d0, p0 = pend.pop(0)
                emit_av(k0, d0, p0)
                # Copy av out to SBUF immediately (releases the av PSUM for
                # the next block ~1.5us after its last AV matmul). Rows
                # 64-127 = softmax denominators, rows 0-63 = unnormalized O.
                rh, cb = hp_stage(hp)
                nc.vector.tensor_copy(dsb[rh, cb:cb + 1024], av[64:128, :])
                nc.vector.tensor_copy(otun[rh, cb:cb + 1024], av[0:64, :])
            emit_norm(qc)
            for stl in range(4):
                for oc in range(2):
                    fq.append((None, op_group(qc, stl, oc)))

        # tail: flush any remaining fillers (out-projections)
        while fq:
            for s in fq.popleft()[1]:
                s()

    nc.compile()
    return nc


def _host_inputs(x, w_qkv, b_qkv, w_o, b_o):
    """Per-core input dicts implementing the sharding + layout prep."""
    tri = np.zeros((128, 128), np.float16)
    for r in range(128):
        tri[r, r:] = 1.0
    M2 = np.concatenate([tri, tri], axis=1)

    in_maps = []
    for c in range(N_CORES):
        b = c // 2
        hs = (c % 2) * HPC
        cols = slice(hs * DH, (hs + HPC) * DH)
        in_maps.append({
            "xT": np.ascontiguousarray(x[b].T).astype(np.float16),
            "wq": w_qkv[:, cols].astype(np.float16),
            "wk": w_qkv[:, D:][:, cols].astype(np.float16),
            "wv": w_qkv[:, 2 * D:][:, cols].astype(np.float16),
            "wo": w_o[hs * DH : (hs + HPC) * DH, :].astype(np.float16),
            "bq": b_qkv[cols].reshape(CH, 1).astype(np.float32),
            "bk": b_qkv[D:][cols].reshape(CH, 1).astype(np.float32),
            "bvb": np.tile(b_qkv[2 * D:][cols].astype(np.float32), (128, 1)),
            "bob": np.tile(b_o.astype(np.float32), (128, 1)),
            "M2": M2,
        })
    return in_maps


def kernel(x, w_qkv, b_qkv, w_o, b_o):
    global _cached
    from concourse.bass_utils import run_bass_kernel_spmd

    x = np.asarray(x)
    w_qkv = np.asarray(w_qkv)
    b_qkv = np.asarray(b_qkv)
    w_o = np.asarray(w_o)
    b_o = np.asarray(b_o)

    if _cached is None:
        _cached = _build_program()
    nc = _cached

    in_maps = _host_inputs(x, w_qkv, b_qkv, w_o, b_o)
    res = run_bass_kernel_spmd(nc, in_maps, list(range(N_CORES)))

    out = np.empty((B, N, D), np.float32)
    for b in range(B):
        out[b] = res.results[2 * b]["y"] + res.results[2 * b + 1]["y"]
    return out


# revision 22
# speedup vs baseline: 1.0088x; 1.0022x over previous
"""Causal self-attention on 8 Trainium2 NeuronCores.

Sharding: core c handles batch b = c//2 and heads [(c%2)*8, (c%2)*8+8).
Each core computes the full QKV projection for its head slice, causal
flash-style attention, and the row-parallel w_o partial product. The two
partials per batch are summed on the host (no device collectives).

All PE matmuls run in fp16 with fp32 PSUM accumulation. Feature-major:
  x^T [D, N]            (host pre-transposed)
  Q^T, K^T [ch, N]      (GEMM with W stationary, x^T moving)
  V [N, 8*(64+64ones)]  (GEMM with x^T stationary, W moving)
  S^T [k, q] = K^T_tile.T @ Q^T  -> exp -> P^T [k, q]
  O^T|denom [128, q] = [V_h|ones64].T @ P^T   (rows 64-127 = denominator)
  y = O^T_norm.T @ W_o  (accumulated over ch tiles)

Causal masking: diagonal-straddling S^T blocks are computed only on
their live column range [delta:512]; the 128-wide triangular boundary
strip of P^T is zeroed post-exp by a DVE multiply with a constant
upper-triangular 0/1 tile. Dead columns are never touched by S, exp,
or AV, so stale PSUM/SBUF there is harmless.

The attention inner loop is exp-bound (ACT ~1.1us/k-tile vs ~0.65us of
PE work), so the QKV projections of future seq-chunks and the previous
chunk's out-projection are emitted as "filler" matmul steps woven into
the attention loop, with deadlines (QKV of chunk sc / channel-tile ct
must precede attention block (qc=sc, hp=ct)).
"""

from collections import deque

import numpy as np

B, N, D, H = 4, 2048, 1024, 16
DH = 64
N_CORES = 8
HPC = 8            # heads per core
CH = HPC * DH      # 512 channels per core
SCALE = 1.0 / 8.0  # 1/sqrt(DH)

_cached = None


def _build_program():
    from contextlib import ExitStack

    import concourse.tile as tile
    from concourse import bacc, mybir

    f16 = mybir.dt.float16
    f32 = mybir.dt.float32
    Exp = mybir.ActivationFunctionType.Exp
    Ln = mybir.ActivationFunctionType.Ln
    mult = mybir.AluOpType.mult
    add = mybir.AluOpType.add

    nc = bacc.Bacc(
        "TRN2", target_bir_lowering=False, debug=False, num_devices=N_CORES
    )

    xT_d = nc.dram_tensor("xT", [D, N], f16, kind="ExternalInput").ap()
    wq_d = nc.dram_tensor("wq", [D, CH], f16, kind="ExternalInput").ap()
    wk_d = nc.dram_tensor("wk", [D, CH], f16, kind="ExternalInput").ap()
    wv_d = nc.dram_tensor("wv", [D, CH], f16, kind="ExternalInput").ap()
    wo_d = nc.dram_tensor("wo", [CH, D], f16, kind="ExternalInput").ap()
    bq_d = nc.dram_tensor("bq", [CH, 1], f32, kind="ExternalInput").ap()
    bk_d = nc.dram_tensor("bk", [CH, 1], f32, kind="ExternalInput").ap()
    bv_d = nc.dram_tensor("bvb", [128, CH], f32, kind="ExternalInput").ap()
    bo_d = nc.dram_tensor("bob", [128, D], f32, kind="ExternalInput").ap()
    M2_d = nc.dram_tensor("M2", [128, 256], f16, kind="ExternalInput").ap()
    y_d = nc.dram_tensor("y", [N, D], f32, kind="ExternalOutput").ap()

    with tile.TileContext(nc) as tc, ExitStack() as ctx:
        const = ctx.enter_context(tc.tile_pool(name="const", bufs=1))
        actp = ctx.enter_context(tc.tile_pool(name="actp", bufs=1))
        work = ctx.enter_context(tc.tile_pool(name="work", bufs=3))
        ps_sp = ctx.enter_context(tc.tile_pool(name="ps_sp", bufs=2, space="PSUM"))
        ps_av = ctx.enter_context(tc.tile_pool(name="ps_av", bufs=1, space="PSUM"))
        ps_fl = ctx.enter_context(tc.tile_pool(name="ps_fl", bufs=2, space="PSUM"))

        # ---- constants / weights into SBUF (4 DMA queues, first-need order)
        wq = [const.tile([128, CH], f16, tag=f"wq{i}", name=f"wq{i}") for i in range(8)]
        wk = [const.tile([128, CH], f16, tag=f"wk{i}", name=f"wk{i}") for i in range(8)]
        wv = [const.tile([128, CH], f16, tag=f"wv{i}", name=f"wv{i}") for i in range(8)]
        xt = [[const.tile([128, 512], f16, tag=f"xt{i}_{sc}", name=f"xt{i}_{sc}")
               for sc in range(4)] for i in range(8)]
        # V[st]: per head h, cols [128h:128h+64] = V_h, [128h+64:128h+128] = 1.0
        V = [actp.tile([128, 1024], f16, tag=f"v{st}", name=f"v{st}")
             for st in range(16)]
        for st in range(16):
            nc.vector.memset(
                V[st][:].rearrange("p (h e) -> p h e", e=128)[:, :, 64:128], 1.0)
        engs = [nc.sync, nc.gpsimd]
        _ei = [0]

        def dma_in(dst, src):
            engs[_ei[0] % len(engs)].dma_start(dst, src)
            _ei[0] += 1

        for i in range(8):
            dma_in(wk[i][:], wk_d[i * 128 : (i + 1) * 128, :])
            dma_in(xt[i][0][:], xT_d[i * 128 : (i + 1) * 128, 0:512])
        bq = [const.tile([128, 1], f32, tag=f"bq{j}", name=f"bq{j}") for j in range(4)]
        bk = [const.tile([128, 1], f32, tag=f"bk{j}", name=f"bk{j}") for j in range(4)]
        for j in range(4):
            dma_in(bq[j][:], bq_d[j * 128 : (j + 1) * 128, :])
            dma_in(bk[j][:], bk_d[j * 128 : (j + 1) * 128, :])
        for i in range(8):
            dma_in(wq[i][:], wq_d[i * 128 : (i + 1) * 128, :])
            dma_in(wv[i][:], wv_d[i * 128 : (i + 1) * 128, :])
        bv_t = const.tile([128, CH], f32, tag="bvb", name="bvb")
        dma_in(bv_t[:], bv_d[:])
        M2_t = const.tile([128, 256], f16, tag="M2", name="M2t")
        dma_in(M2_t[:], M2_d[:])
        for sc in range(1, 4):
            for i in range(8):
                dma_in(xt[i][sc][:],
                       xT_d[i * 128 : (i + 1) * 128, sc * 512 : (sc + 1) * 512])
        wo = [const.tile([128, D], f16, tag=f"wo{j}", name=f"wo{j}") for j in range(4)]
        for j in range(4):
            dma_in(wo[j][:], wo_d[j * 128 : (j + 1) * 128, :])
        bo_t = const.tile([128, D], f32, tag="bob", name="bob")
        dma_in(bo_t[:], bo_d[:])

        # ---- persistent activations ----
        QT = [[actp.tile([128, 512], f16, tag=f"qt{ct}_{sc}", name=f"qt{ct}_{sc}")
               for sc in range(4)] for ct in range(4)]
        KT = [[actp.tile([128, 512], f16, tag=f"kt{ct}_{sc}", name=f"kt{ct}_{sc}")
               for sc in range(4)] for ct in range(4)]
        OTn = [[actp.tile([128, 512], f16, tag=f"otn{hp}_{qc}", name=f"otn{hp}_{qc}")
                for qc in range(4)] for hp in range(4)]
        # per-qc staging for deferred softmax normalization: head-pair hp ->
        # rows 64*(hp//2):+64, cols (hp%2)*1024:+1024
        dsb = actp.tile([128, 2048], f16, tag="dsb", name="dsb")
        otun = actp.tile([128, 2048], f16, tag="otun", name="otun")

        def hp_stage(hp):
            rh = slice(64 * (hp // 2), 64 * (hp // 2) + 64)
            cb = (hp % 2) * 1024
            return rh, cb

        def emit_norm(qc):
            # 1/d = exp(-ln d) batched over all 4 head-pairs of chunk qc:
            # two ACT instructions total (one table swap pair per chunk).
            lnq = work.tile([128, 2048], f32, tag="lnq", name="lnq", bufs=1)
            rq = work.tile([128, 2048], f16, tag="rq", name="rq", bufs=1)
            nc.scalar.activation(lnq[:], dsb[:], Ln)
            nc.scalar.activation(rq[:], lnq[:], Exp, scale=-1.0)
            for hp in range(4):
                rh, cb = hp_stage(hp)
                nc.vector.tensor_mul(OTn[hp][qc][0:64, :],
                                     otun[rh, cb:cb + 512],
                                     rq[rh, cb:cb + 512])
                nc.vector.tensor_mul(OTn[hp][qc][64:128, :],
                                     otun[rh, cb + 512:cb + 1024],
                                     rq[rh, cb + 512:cb + 1024])

        # ---- filler machinery ----------------------------------------
        # Each filler entry: (deadline, steps). deadline = (qc, hp) block
        # index before which all steps must be emitted; steps are closures
        # emitting ~one 512-col matmul (~215ns of PE) each.
        fq = deque()
        credit = [0.0]

        def kt_group(ct, sc):
            p = ps_fl.tile([128, 512], f32, tag="fl", name="pfl")
            steps = []
            for d in range(8):
                steps.append(lambda d=d, p=p, ct=ct, sc=sc: nc.tensor.matmul(
                    p[:], wk[d][:, ct * 128:(ct + 1) * 128], xt[d][sc][:],
                    start=(d == 0), stop=(d == 7), skip_group_check=True))
            steps.append(lambda p=p, ct=ct, sc=sc: nc.vector.tensor_scalar_add(
                KT[ct][sc][:], p[:], bk[ct][:]))
            return steps

        def qt_group(ct, sc):
            p = ps_fl.tile([128, 512], f32, tag="fl", name="pfl")
            steps = []
            for d in range(8):
                steps.append(lambda d=d, p=p, ct=ct, sc=sc: nc.tensor.matmul(
                    p[:], wq[d][:, ct * 128:(ct + 1) * 128], xt[d][sc][:],
                    start=(d == 0), stop=(d == 7), skip_group_check=True))
            steps.append(lambda p=p, ct=ct, sc=sc: nc.vector.tensor_scalar_add(
                QT[ct][sc][:], p[:], bq[ct][:]))
            return steps

        def v_group(stl, sc):
            st = 4 * sc + stl
            ts = slice(stl * 128, (stl + 1) * 128)
            p = ps_fl.tile([128, 512], f32, tag="fl", name="pfl")
            steps = []
            for d in range(8):
                steps.append(lambda d=d, p=p, sc=sc, ts=ts: nc.tensor.matmul(
                    p[:], xt[d][sc][:, ts], wv[d][:, :],
                    start=(d == 0), stop=(d == 7), skip_group_check=True))

            def fin(p=p, st=st):
                nc.vector.scalar_tensor_tensor(
                    V[st][:].rearrange("p (h e) -> p h e", e=128)[:, :, 0:64],
                    p[:].rearrange("p (h e) -> p h e", e=64),
                    1.0,
                    bv_t[:].rearrange("p (h e) -> p h e", e=64),
                    mult, add,
                )
            steps.append(fin)
            return steps

        def op_group(qc, stl, oc):
            st = 4 * qc + stl
            sl = slice(stl * 128, (stl + 1) * 128)
            ocs = slice(oc * 512, (oc + 1) * 512)
            yp = ps_fl.tile([128, 512], f32, tag="fl", name="yfl")
            steps = []
            for hpp in range(4):
                steps.append(lambda hpp=hpp, yp=yp, qc=qc, sl=sl, ocs=ocs:
                             nc.tensor.matmul(
                                 yp[:], OTn[hpp][qc][:, sl], wo[hpp][:, ocs],
                                 start=(hpp == 0), stop=(hpp == 3),
                                 skip_group_check=True))

            def fin(yp=yp, st=st, ocs=ocs):
                ysb = work.tile([128, 512], f32, tag="ysb", name="ysb")
                nc.vector.scalar_tensor_tensor(ysb[:], yp[:], 1.0,
                                               bo_t[:, ocs], mult, add)
                nc.sync.dma_start(y_d[st * 128:(st + 1) * 128, ocs], ysb[:])
            steps.append(fin)
            return steps

        def enqueue_qkv(sc):
            fq.append(((sc, 0, 0), kt_group(0, sc)))
            fq.append(((sc, 0, 0), qt_group(0, sc)))
            for stl in range(4):
                # V[4sc+stl] first consumed by AV(kt=4sc+stl) of block (sc,0),
                # which is emitted at iter kt+1 (or post-loop for the last kt)
                fq.append(((sc, 0, 4 * sc + stl + 1), v_group(stl, sc)))
            for ct in range(1, 4):
                fq.append(((sc, ct, 0), kt_group(ct, sc)))
                fq.append(((sc, ct, 0), qt_group(ct, sc)))

        def flush(block):
            # emit all filler steps whose deadline <= current block;
            # deadline-None (out-projection) entries stay, order preserved
            kept = []
            while fq:
                dl, steps = fq.popleft()
                if dl is not None and dl <= block:
                    for s in steps:
                        s()
                else:
                    kept.append((dl, steps))
            fq.extend(kept)

        def pop_steps():
            while credit[0] >= 215.0 and fq:
                dl, steps = fq[0]
                steps.pop(0)()
                if not steps:
                    fq.popleft()
                credit[0] -= 215.0

        # ---- attention blocks with woven fillers ----------------------
        enqueue_qkv(0)
        op_hold = []

        for qc in range(4):
            if qc < 3:
                enqueue_qkv(qc + 1)
            else:
                fq.extend(op_hold)
                op_hold.clear()
            nkt = 4 * (qc + 1)
            for hp in range(4):
                flush((qc, hp, 0))
                h0, h1 = 2 * hp, 2 * hp + 1
                av = ps_av.tile([128, 1024], f32, tag="av", name="av")
                pend = []  # (kt, delta, pt) awaiting AV

                def emit_av(kt, delta, pt, av=av, h0=h0, h1=h1, nkt=nkt):
                    first, last = kt == 0, kt == nkt - 1
                    nc.tensor.matmul(
                        av[:, delta:512], V[kt][:, h0 * 128: h0 * 128 + 128],
                        pt[:, delta:512],
                        start=first, stop=last, skip_group_check=True)
                    nc.tensor.matmul(
                        av[:, 512 + delta:1024], V[kt][:, h1 * 128: h1 * 128 + 128],
                        pt[:, 512 + delta:1024],
                        start=first, stop=last, skip_group_check=True)

                for kt in range(nkt):
                    flush((qc, hp, kt))
                    diag = kt >= 4 * qc
                    delta = 128 * kt - 512 * qc if diag else 0
                    sp = ps_sp.tile([128, 1024], f32, tag="sp", name="sp")
                    kcol = slice((kt % 4) * 128, (kt % 4) * 128 + 128)
                    nc.tensor.matmul(
                        sp[:, delta:512], KT[hp][kt // 4][0:64, kcol],
                        QT[hp][qc][0:64, delta:512],
                        start=True, stop=True, skip_group_check=True)
                    nc.tensor.matmul(
                        sp[:, 512 + delta:1024], KT[hp][kt // 4][64:128, kcol],
                        QT[hp][qc][64:128, delta:512],
                        start=True, stop=True, skip_group_check=True)
                    pt = work.tile([128, 1024], f16, tag="pt", name="pt", bufs=4)
                    if delta:
                        sp3 = sp[:].rearrange("p (b c) -> p b c", c=512)[:, :, delta:512]
                        pt3 = pt[:].rearrange("p (b c) -> p b c", c=512)[:, :, delta:512]
                        nc.scalar.activation(pt3, sp3, Exp, scale=SCALE)
                    else:
                        nc.scalar.activation(pt[:], sp[:], Exp, scale=SCALE)
                    if diag:
                        ptm = pt[:].rearrange("p (b c) -> p b c", c=512)[
                            :, :, delta:delta + 128]
                        m3 = M2_t[:].rearrange("p (b c) -> p b c", c=128)
                        nc.vector.tensor_tensor(ptm, ptm, m3, mult)
                    pend.append((kt, delta, pt))
                    live = 2 * (512 - delta)
                    credit[0] = min(credit[0] + (264 + 0.83 * live)
                                    - (80 + 0.625 * live), 8000.0)
                    if len(pend) > 1:
                        k0, d0, p0 = pend.pop(0)
                        emit_av(k0, d0, p0)
                    pop_steps()
                flush((qc, hp, nkt))
                credit[0] = min(credit[0] + 2000.0, 8000.0)
                pop_steps()
                k0, d0, p0 = pend.pop(0)
                emit_av(k0, d0, p0)
                # Copy av out to SBUF immediately (releases the av PSUM for
                # the next block ~1.5us after its last AV matmul). Rows
                # 64-127 = softmax denominators, rows 0-63 = unnormalized O.
                rh, cb = hp_stage(hp)
                nc.vector.tensor_copy(dsb[rh, cb:cb + 1024], av[64:128, :])
                nc.vector.tensor_copy(otun[rh, cb:cb + 1024], av[0:64, :])
            emit_norm(qc)
            for stl in range(4):
                for oc in range(2):
                    entry = (None, op_group(qc, stl, oc))
                    if qc < 2:
                        op_hold.append(entry)
                    else:
                        fq.append(entry)

        # tail: flush any remaining fillers (out-projections)
        while fq:
            for s in fq.popleft()[1]:
                s()

    nc.compile()
    return nc


def _host_inputs(x, w_qkv, b_qkv, w_o, b_o):
    """Per-core input dicts implementing the sharding + layout prep."""
    tri = np.zeros((128, 128), np.float16)
    for r in range(128):
        tri[r, r:] = 1.0
    M2 = np.concatenate([tri, tri], axis=1)

    in_maps = []
    for c in range(N_CORES):
        b = c // 2
        hs = (c % 2) * HPC
        cols = slice(hs * DH, (hs + HPC) * DH)
        in_maps.append({
            "xT": np.ascontiguousarray(x[b].T).astype(np.float16),
            "wq": w_qkv[:, cols].astype(np.float16),
            "wk": w_qkv[:, D:][:, cols].astype(np.float16),
            "wv": w_qkv[:, 2 * D:][:, cols].astype(np.float16),
            "wo": w_o[hs * DH : (hs + HPC) * DH, :].astype(np.float16),
            "bq": b_qkv[cols].reshape(CH, 1).astype(np.float32),
            "bk": b_qkv[D:][cols].reshape(CH, 1).astype(np.float32),
            "bvb": np.tile(b_qkv[2 * D:][cols].astype(np.float32), (128, 1)),
            "bob": np.tile(b_o.astype(np.float32), (128, 1)),
            "M2": M2,
        })
    return in_maps


def kernel(x, w_qkv, b_qkv, w_o, b_o):
    global _cached
    from concourse.bass_utils import run_bass_kernel_spmd

    x = np.asarray(x)
    w_qkv = np.asarray(w_qkv)
    b_qkv = np.asarray(b_qkv)
    w_o = np.asarray(w_o)
    b_o = np.asarray(b_o)

    if _cached is None:
        _cached = _build_program()
    nc = _cached

    in_maps = _host_inputs(x, w_qkv, b_qkv, w_o, b_o)
    res = run_bass_kernel_spmd(nc, in_maps, list(range(N_CORES)))

    out = np.empty((B, N, D), np.float32)
    for b in range(B):
        out[b] = res.results[2 * b]["y"] + res.results[2 * b + 1]["y"]
    return out
